# revision 15
# baseline (speedup 1.0000x reference)
"""Trainium2 Bass kernel for a dense transformer block (pre-norm, GQA+RoPE
attention, squared-ReLU FFN), sharded over 8 NeuronCores.

Sharding: tokens (B*S = 4096) are split into 8 contiguous slices of 512; core
c handles batch c//4, tokens [512*(c%4), 512*(c%4)+512).  Each core computes
q/k/v for its own tokens, k/v are exchanged with the 3 other cores of the
same batch via a grouped AllGather, and each core then runs full attention
for its queries plus the whole FFN for its tokens.  All activations are kept
feature-major ([feature, token]) on-chip so every matmul contracts over the
partition axis; the host passes x pre-transposed and re-transposes the
output.

Matmuls run as float32r (full PE rate at free-dim >= 256); every tensor that
feeds a float32r matmul is produced with a float32r-rounding output.  The
attention mask is applied multiplicatively after exp() using host-precomputed
exp(mask) tiles, which keeps the program identical on every core (SPMD) for
any mask.
"""

import sys

sys.path.insert(0, "/opt/trn_rl_repo")

import numpy as np

# problem shapes (hardcoded per contract)
B, S, D = 2, 2048, 2048
QC, KVC, HD = 16, 4, 128
FF = 8192
EPS = 1e-5
NCORES = 8
T = 512                    # tokens per core
NT = T // 128              # 4 token tiles per core
NKD = D // 128             # 16 contraction tiles over D
NFT = (D + 2 * KVC * HD) // 128   # 24 qkv feature tiles
NTK = S // 128             # 16 key tiles
NKF = FF // 128            # 64 ffn feature tiles
KFG = 8                    # kf tiles per ffn group
NG = NKF // KFG            # 8 ffn groups

_CACHE = {}
LAST_RESULTS = None


def _build_program():
    import concourse.bass as bass  # noqa: F401
    import concourse.mybir as mybir
    import concourse.tile as tile
    from concourse import bacc
    from concourse.masks import make_identity

    dt = mybir.dt
    f32 = dt.float32
    f32r = dt.float32r
    bf16 = dt.bfloat16
    AF = mybir.ActivationFunctionType
    ALU = mybir.AluOpType

    nc = bacc.Bacc("TRN2", target_bir_lowering=False, debug=False,
                   num_devices=NCORES)

    # ---------------- kernel I/O ----------------
    xT = nc.dram_tensor("xT", [D, T], f32, kind="ExternalInput")
    wqkvb = nc.dram_tensor("wqkvb", [NFT, 128, NKD, 128], f32r, kind="ExternalInput")
    wob = nc.dram_tensor("wob", [NKD, 128, NKD, 128], f32r, kind="ExternalInput")
    wkb = nc.dram_tensor("wkb", [NKF, 128, NKD, 128], f32r, kind="ExternalInput")
    wvb = nc.dram_tensor("wvb", [NKD, NG, 128, KFG, 128], f32r, kind="ExternalInput")
    sinq = nc.dram_tensor("sinq", [HD, T], f32, kind="ExternalInput")
    cosq = nc.dram_tensor("cosq", [HD, T], f32, kind="ExternalInput")
    sink = nc.dram_tensor("sink", [HD, T], f32, kind="ExternalInput")
    cosk = nc.dram_tensor("cosk", [HD, T], f32, kind="ExternalInput")
    rotm = nc.dram_tensor("rotm", [HD, HD], f32r, kind="ExternalInput")
    em = nc.dram_tensor("em", [128, NTK, T], dt.bfloat16, kind="ExternalInput")
    yT = nc.dram_tensor("yT", [D, T], f32, kind="ExternalOutput")

    with tile.TileContext(nc) as tc:
        # pools that live for the whole kernel
        pc = tc.alloc_tile_pool(name="pconst", bufs=1)
        pp = tc.alloc_tile_pool(name="ppsum", bufs=1, space="PSUM")
        pd = tc.alloc_tile_pool(name="pdram", bufs=1, space="DRAM")
        pst = tc.alloc_tile_pool(name="pstat", bufs=2)
        pxn = tc.alloc_tile_pool(name="pxn", bufs=1)

        def mm_ps(name):
            return pp.tile([128, T], f32, tag="mm", bufs=4, name=name)

        # ---------------- constants ----------------
        ones32 = pc.tile([128, 1], f32, name="ones32")
        nc.vector.memset(ones32[:], 1.0)
        eps_sb = pc.tile([1, 1], f32, name="eps_sb")
        nc.vector.memset(eps_sb[:], EPS)
        onesr = pc.tile([128, 1], f32r, name="onesr")
        nc.vector.tensor_copy(onesr[:], ones32[:])

        def rmsnorm(src_sb, dst_sb, label):
            """dst = src * rsqrt(mean_d(src^2) + eps); src/dst [128, NKD, T]."""
            pwn = tc.alloc_tile_pool(name=f"pnorm_{label}", bufs=3)
            ss_ps = pp.tile([1, T], f32, tag="st", bufs=2, name=f"ss_{label}")
            for kd in range(NKD):
                xsq = pwn.tile([128, T], f32, tag="xsq", name=f"xsq_{label}_{kd}")
                nc.scalar.activation(xsq[:], src_sb[:, kd, :], AF.Square)
                # full-precision fp32 matmul for the cross-partition sum
                nc.tensor.matmul(ss_ps[:], ones32[:], xsq[:],
                                 start=(kd == 0), stop=(kd == NKD - 1))
            s_sb = pst.tile([1, T], f32, tag="s1", name=f"s_{label}")
            nc.scalar.activation(s_sb[:], ss_ps[:], AF.Sqrt,
                                 scale=1.0 / D, bias=eps_sb[:])
            rs_sb = pst.tile([1, T], f32, tag="s2", name=f"rs_{label}")
            nc.vector.reciprocal(rs_sb[:], s_sb[:])
            rsb = pst.tile([128, T], f32, tag="s3", name=f"rsb_{label}")
            nc.gpsimd.partition_broadcast(rsb[:], rs_sb[:])
            for kd in range(NKD):
                nc.vector.tensor_tensor(dst_sb[:, kd, :], src_sb[:, kd, :],
                                        rsb[:], ALU.mult)
            pwn.release()

        # ---------------- phase 0/1: load x, norm1 ----------------
        px = tc.alloc_tile_pool(name="px", bufs=1)
        xT_sb = px.tile([128, NKD, T], f32, name="xT_sb")
        nc.sync.dma_start(xT_sb[:], xT.rearrange("(kd p) t -> p kd t", p=128))

        xn_sb = pxn.tile([128, NKD, T], f32r, name="xn_sb")
        rmsnorm(xT_sb, xn_sb, "n1")
        px.release()  # xT_sb dead (re-read from DRAM later)

        # ---------------- phase 2: qkv projection + rope + AllGather ----------
        pq = tc.alloc_tile_pool(name="pq", bufs=1)
        q_sb = pq.tile([128, QC, T], f32r, name="q_sb")

        pkva = tc.alloc_tile_pool(name="pkva", bufs=1)
        kall_sb = pkva.tile([128, KVC, NTK, 128], f32r, name="kall_sb")
        vall_sb = pkva.tile([128, KVC, NTK, 128], f32r, name="vall_sb")

        pkv = tc.alloc_tile_pool(name="pkv", bufs=1)
        ko_sb = pkv.tile([128, KVC, T], f32r, name="ko_sb")        # rope'd+scaled k
        vo_sb = pkv.tile([128, KVC, NT, 128], f32r, name="vo_sb")  # v token-major
        idn = pkv.tile([128, 128], f32, name="idn")
        make_identity(nc, idn)
        rot_sb = pkv.tile([HD, HD], f32r, name="rot_sb")
        nc.sync.dma_start(rot_sb[:], rotm[:])
        sinq_sb = pkv.tile([HD, T], f32, name="sinq_sb")
        nc.sync.dma_start(sinq_sb[:], sinq[:])
        cosq_sb = pkv.tile([HD, T], f32, name="cosq_sb")
        nc.sync.dma_start(cosq_sb[:], cosq[:])
        sink_sb = pkv.tile([HD, T], f32, name="sink_sb")
        nc.sync.dma_start(sink_sb[:], sink[:])
        cosk_sb = pkv.tile([HD, T], f32, name="cosk_sb")
        nc.sync.dma_start(cosk_sb[:], cosk[:])

        kv_in = pd.tile([2 * KVC, 128, T], f32r, name="kv_in")
        kv_all = pd.tile([4, 2 * KVC, 128, T], f32r, name="kv_all")

        pwq = tc.alloc_tile_pool(name="pwqkv", bufs=3)

        def rope_apply(raw_ps, sin_t, cos_t, dst_ap, label):
            """dst = raw*cos + rot(raw)*sin, raw in PSUM [128, T]."""
            raw_sb = pwq.tile([128, T], f32r, tag="rraw", name=f"rr_{label}")
            nc.scalar.activation(raw_sb[:], raw_ps[:], AF.Copy)
            rot_ps = mm_ps(f"rot_{label}")
            nc.tensor.matmul(rot_ps[:], rot_sb[:], raw_sb[:],
                             start=True, stop=True)
            mcos = pwq.tile([128, T], f32, tag="mcos", name=f"mc_{label}")
            nc.vector.tensor_tensor(mcos[:], raw_sb[:], cos_t[:], ALU.mult)
            msin = pwq.tile([128, T], f32, tag="msin", name=f"ms_{label}")
            nc.vector.tensor_tensor(msin[:], rot_ps[:], sin_t[:], ALU.mult)
            nc.vector.tensor_tensor(dst_ap, mcos[:], msin[:], ALU.add)

        # feature-tile order: k heads, v heads, then q heads, so the
        # AllGather can launch while q is still being projected.
        ft_order = [16 + h for h in range(KVC)] + [20 + h for h in range(KVC)] \
            + list(range(QC))
        for ft in ft_order:
            wsb = pwq.tile([128, NKD, 128], f32r, tag="wqkv", bufs=2,
                           name=f"wq_{ft}")
            nc.sync.dma_start(wsb[:], wqkvb[ft])
            acc = mm_ps(f"qkv_{ft}")
            for kd in range(NKD):
                nc.tensor.matmul(acc[:], wsb[:, kd, :], xn_sb[:, kd, :],
                                 start=(kd == 0), stop=(kd == NKD - 1))
            if 16 <= ft < 20:      # k head
                h = ft - 16
                rope_apply(acc, sink_sb, cosk_sb, ko_sb[:, h, :], f"k{h}")
            elif ft >= 20:         # v head
                h = ft - 20
                vtmp = pwq.tile([128, T], f32, tag="vtmp", bufs=2,
                                name=f"vt_{h}")
                nc.scalar.activation(vtmp[:], acc[:], AF.Copy)
                for j in range(NT):
                    vt_ps = pp.tile([128, 128], f32, tag="mm", bufs=4,
                                    name=f"vtp_{h}_{j}")
                    nc.tensor.transpose(vt_ps[:], vtmp[:, j * 128:(j + 1) * 128],
                                        idn[:])
                    nc.vector.tensor_copy(vo_sb[:, h, j, :], vt_ps[:])
            else:                  # q head
                h = ft
                rope_apply(acc, sinq_sb, cosq_sb, q_sb[:, h, :], f"q{h}")

        # ship own k/v to DRAM bounce, AllGather within the 4-core batch group
        for h in range(KVC):
            nc.sync.dma_start(kv_in[h], ko_sb[:, h, :])
            nc.sync.dma_start(kv_in[KVC + h].rearrange("p (j f) -> p j f", j=NT),
                              vo_sb[:, h, :, :])
        nc.gpsimd.collective_compute(
            "AllGather", ALU.bypass,
            replica_groups=[[0, 1, 2, 3], [4, 5, 6, 7]],
            ins=[kv_in.opt()], outs=[kv_all.opt()],
        )
        for h in range(KVC):
            for r in range(4):
                nc.sync.dma_start(
                    kall_sb[:, h, 4 * r:4 * (r + 1), :],
                    kv_all[r, h].rearrange("p (j f) -> p j f", j=NT))
                nc.sync.dma_start(
                    vall_sb[:, h, 4 * r:4 * (r + 1), :],
                    kv_all[r, KVC + h].rearrange("p (j f) -> p j f", j=NT))
        pwq.release()
        pkv.release()

        # ---------------- phase 3: attention ----------------
        po = tc.alloc_tile_pool(name="po", bufs=1, side="right")
        o_sb = po.tile([128, QC, T], f32r, name="o_sb")

        pem = tc.alloc_tile_pool(name="pem", bufs=1)
        em_sb = pem.tile([128, NTK, T], bf16, name="em_sb")
        nc.sync.dma_start(em_sb[:], em[:])

        pwa = tc.alloc_tile_pool(name="pwat", bufs=3)
        for h in range(QC):
            kvh = h % KVC
            o_ps = pp.tile([128, T], f32, tag="acc", bufs=2, name=f"o_ps_{h}")
            den_ps = pp.tile([1, T], f32, tag="st", bufs=2, name=f"den_{h}")
            for j in range(NTK):
                s_ps = mm_ps(f"s_{h}_{j}")
                nc.tensor.matmul(s_ps[:], kall_sb[:, kvh, j, :], q_sb[:, h, :],
                                 start=True, stop=True)
                e_sb = pwa.tile([128, T], f32, tag="exp", name=f"e_{h}_{j}")
                nc.scalar.activation(e_sb[:], s_ps[:], AF.Exp)
                p_sb = pwa.tile([128, T], f32r, tag="pm", name=f"p_{h}_{j}")
                nc.vector.tensor_tensor(p_sb[:], e_sb[:], em_sb[:, j, :],
                                        ALU.mult)
                nc.tensor.matmul(o_ps[:], vall_sb[:, kvh, j, :], p_sb[:],
                                 start=(j == 0), stop=(j == NTK - 1))
                nc.tensor.matmul(den_ps[:], onesr[:], p_sb[:],
                                 start=(j == 0), stop=(j == NTK - 1))
            rden = pst.tile([1, T], f32, tag="s1", name=f"rden_{h}")
            nc.vector.reciprocal(rden[:], den_ps[:])
            rdb = pst.tile([128, T], f32, tag="s3", name=f"rdb_{h}")
            nc.gpsimd.partition_broadcast(rdb[:], rden[:])
            nc.vector.tensor_tensor(o_sb[:, h, :], o_ps[:], rdb[:], ALU.mult)
        pwa.release()
        pem.release()
        pkva.release()
        pq.release()

        # ---------------- phase 4: wo projection + residual ----------------
        py1 = tc.alloc_tile_pool(name="py1", bufs=1)
        y1_sb = py1.tile([128, NKD, T], f32, name="y1_sb")
        pwo = tc.alloc_tile_pool(name="pwwo", bufs=3)
        for dtile in range(NKD):
            wsb = pwo.tile([128, NKD, 128], f32r, tag="wo", name=f"wo_{dtile}")
            nc.sync.dma_start(wsb[:], wob[dtile])
            a_ps = mm_ps(f"wo_ps_{dtile}")
            for kd in range(NKD):
                nc.tensor.matmul(a_ps[:], wsb[:, kd, :], o_sb[:, kd, :],
                                 start=(kd == 0), stop=(kd == NKD - 1))
            xre = pwo.tile([128, T], f32, tag="xre", bufs=2, name=f"xre_{dtile}")
            nc.sync.dma_start(xre[:], xT[128 * dtile:128 * (dtile + 1), :])
            nc.vector.tensor_tensor(y1_sb[:, dtile, :], a_ps[:], xre[:], ALU.add)
        pwo.release()
        po.release()

        # ---------------- phase 5: norm2 ----------------
        xn2_sb = xn_sb  # reuse the same SBUF tile
        rmsnorm(y1_sb, xn2_sb, "n2")

        # ---------------- phase 6: FFN ----------------
        pf = tc.alloc_tile_pool(name="pffn", bufs=1)
        y2a_sb = pf.tile([128, NKD, T], f32, name="y2a_sb")
        y2b_sb = pf.tile([128, NKD, T], f32, name="y2b_sb")
        pff = tc.alloc_tile_pool(name="pwff", bufs=3)
        for g in range(NG):
            h_tiles = []
            for kf8 in range(KFG):
                kf = g * KFG + kf8
                wkt = pff.tile([128, NKD, 128], f32r, tag="wk", bufs=2,
                               name=f"wk_{kf}")
                nc.sync.dma_start(wkt[:], wkb[kf])
                h_ps = mm_ps(f"h_ps_{kf}")
                for kd in range(NKD):
                    nc.tensor.matmul(h_ps[:], wkt[:, kd, :], xn2_sb[:, kd, :],
                                     start=(kd == 0), stop=(kd == NKD - 1))
                hr = pf.tile([128, T], f32r, tag=f"h{kf8}", bufs=2,
                             name=f"hr_{kf}")
                nc.scalar.activation(hr[:], h_ps[:], AF.Relu)
                # in-place square: relu(x)^2
                nc.vector.tensor_tensor(hr[:], hr[:], hr[:], ALU.mult)
                h_tiles.append(hr)
            src = y1_sb if g == 0 else (y2a_sb if g % 2 == 1 else y2b_sb)
            dst = y2a_sb if g % 2 == 0 else y2b_sb
            for dtile in range(NKD):
                wvt = pff.tile([128, KFG, 128], f32r, tag="wv", bufs=3,
                               name=f"wv_{g}_{dtile}")
                nc.sync.dma_start(wvt[:], wvb[dtile, g])
                y2_ps = mm_ps(f"y2_ps_{g}_{dtile}")
                for kf8 in range(KFG):
                    nc.tensor.matmul(y2_ps[:], wvt[:, kf8, :], h_tiles[kf8][:],
                                     start=(kf8 == 0), stop=(kf8 == KFG - 1))
                nc.vector.tensor_tensor(dst[:, dtile, :], y2_ps[:],
                                        src[:, dtile, :], ALU.add)
        final = y2a_sb if (NG - 1) % 2 == 0 else y2b_sb
        for dtile in range(NKD):
            nc.sync.dma_start(yT[128 * dtile:128 * (dtile + 1), :],
                              final[:, dtile, :])

        pff.release()
        pf.release()
        py1.release()
        pxn.release()
        pst.release()
        pd.release()
        pp.release()
        pc.release()

    nc.compile()
    return nc


def _prep_inputs(inputs):
    """Host-side folding/layout; returns per-core in_maps."""
    x = np.asarray(inputs["x"], np.float32)
    mask = np.asarray(inputs["mask"], np.float32)
    rsin = np.asarray(inputs["rope_sin"], np.float32)
    rcos = np.asarray(inputs["rope_cos"], np.float32)
    wqkv = np.asarray(inputs["wqkv"], np.float32)
    wo = np.asarray(inputs["wo"], np.float32)
    n1 = np.asarray(inputs["norm1_w"], np.float32)
    n2 = np.asarray(inputs["norm2_w"], np.float32)
    wk = np.asarray(inputs["ffn_wk"], np.float32)
    wv = np.asarray(inputs["ffn_wv"], np.float32)

    wqkv_f = wqkv * n1[:, None]
    wk_f = wk * n2[:, None]
    wqkv_b = np.ascontiguousarray(
        wqkv_f.reshape(NKD, 128, NFT, 128).transpose(2, 1, 0, 3))
    wo_b = np.ascontiguousarray(
        wo.reshape(NKD, 128, NKD, 128).transpose(2, 1, 0, 3))
    wk_b = np.ascontiguousarray(
        wk_f.reshape(NKD, 128, NKF, 128).transpose(2, 1, 0, 3))
    wv_b = np.ascontiguousarray(
        wv.reshape(NG, KFG, 128, NKD, 128).transpose(3, 0, 2, 1, 4))

    scale = HD ** -0.5
    sinT = np.ascontiguousarray(rsin.T)          # [HD, S]
    cosT = np.ascontiguousarray(rcos.T)
    sinTk = sinT * scale
    cosTk = cosT * scale

    R = np.zeros((HD, HD), np.float32)
    idx = np.arange(0, HD, 2)
    R[idx, idx + 1] = -1.0
    R[idx + 1, idx] = 1.0
    rotm_h = np.ascontiguousarray(R.T)

    import ml_dtypes
    with np.errstate(under="ignore", over="ignore"):
        emfull = np.exp(mask).astype(np.float32)   # [tq, tk]

    in_maps = []
    for c in range(NCORES):
        bc, sc = c // 4, c % 4
        tok = slice(T * sc, T * (sc + 1))
        xTc = np.ascontiguousarray(x[bc, tok].T)
        em_c = np.ascontiguousarray(
            emfull[tok, :].T.reshape(NTK, 128, T).transpose(1, 0, 2)
        ).astype(ml_dtypes.bfloat16)
        in_maps.append(dict(
            xT=xTc, wqkvb=wqkv_b, wob=wo_b, wkb=wk_b, wvb=wv_b,
            sinq=np.ascontiguousarray(sinT[:, tok]),
            cosq=np.ascontiguousarray(cosT[:, tok]),
            sink=np.ascontiguousarray(sinTk[:, tok]),
            cosk=np.ascontiguousarray(cosTk[:, tok]),
            rotm=rotm_h, em=em_c,
        ))
    return in_maps


def _make_runner(nc):
    """Mirror of bass2jax.run_bass_via_pjrt's multi-core path, but reusable:
    the jitted executable and device-resident inputs persist across calls so
    repeat executions skip host->device transfer (for steady-state timing)."""
    import jax
    import jax.numpy as jnp
    from jax.sharding import Mesh, PartitionSpec
    from jax.experimental.shard_map import shard_map
    import concourse.mybir as mybir
    from concourse.bass2jax import (
        _bass_exec_p, install_neuronx_cc_hook, partition_id_tensor)

    install_neuronx_cc_hook()
    partition_name = nc.partition_id_tensor.name if nc.partition_id_tensor else None

    in_names, out_names, out_avals = [], [], []
    for alloc in nc.m.functions[0].allocations:
        if not isinstance(alloc, mybir.MemoryLocationSet):
            continue
        name = alloc.memorylocations[0].name
        if alloc.kind == "ExternalInput":
            if name != partition_name:
                in_names.append(name)
        elif alloc.kind == "ExternalOutput":
            out_names.append(name)
            out_avals.append(jax.core.ShapedArray(
                tuple(alloc.tensor_shape), mybir.dt.np(alloc.dtype)))
    n_params = len(in_names)
    all_names = list(in_names) + list(out_names)
    if partition_name is not None:
        all_names.append(partition_name)

    def _body(*args):
        operands = list(args)
        if partition_name is not None:
            operands.append(partition_id_tensor())
        return tuple(_bass_exec_p.bind(
            *operands,
            out_avals=tuple(out_avals),
            in_names=tuple(all_names),
            out_names=tuple(out_names),
            lowering_input_output_aliases=(),
            sim_require_finite=True,
            sim_require_nnan=True,
            nc=nc,
        ))

    devices = jax.devices()[:NCORES]
    mesh = Mesh(np.asarray(devices), ("core",))
    n_outs = len(out_names)
    donate = tuple(range(n_params, n_params + n_outs))
    sharded = jax.jit(
        shard_map(_body, mesh=mesh,
                  in_specs=(PartitionSpec("core"),) * (n_params + n_outs),
                  out_specs=(PartitionSpec("core"),) * n_outs,
                  check_rep=False),
        donate_argnums=donate, keep_unused=True)

    in_shardings = jax.sharding.NamedSharding(mesh, PartitionSpec("core"))

    @jax.jit
    def _mkzeros():
        return tuple(
            jnp.zeros((NCORES * av.shape[0], *av.shape[1:]), av.dtype)
            for av in out_avals)
    mkzeros = jax.jit(_mkzeros, out_shardings=(in_shardings,) * n_outs)

    state = {}

    def put_inputs(in_maps):
        concat = [np.concatenate([np.asarray(in_maps[c][nm])
                                  for c in range(NCORES)], axis=0)
                  for nm in in_names]
        state["dev_in"] = [jax.device_put(a, in_shardings) for a in concat]
        jax.block_until_ready(state["dev_in"])

    def run(download=True):
        zeros = mkzeros()
        outs = sharded(*state["dev_in"], *zeros)
        jax.block_until_ready(outs)
        if not download:
            return None
        per_core = []
        for c in range(NCORES):
            per_core.append({
                nm: np.asarray(outs[i]).reshape(NCORES, *out_avals[i].shape)[c]
                for i, nm in enumerate(out_names)})
        return per_core

    return put_inputs, run


def get_runner():
    if "nc" not in _CACHE:
        _CACHE["nc"] = _build_program()
    if "runner" not in _CACHE:
        _CACHE["runner"] = _make_runner(_CACHE["nc"])
    return _CACHE["runner"]


def kernel(**inputs) -> np.ndarray:
    put_inputs, run = get_runner()
    in_maps = _prep_inputs(inputs)
    put_inputs(in_maps)
    results = run(download=True)

    y = np.empty((B, S, D), np.float32)
    for c in range(NCORES):
        bc, sc = c // 4, c % 4
        y[bc, T * sc:T * (sc + 1), :] = results[c]["yT"].T
    return y


# ---------------------------------------------------------------------------
# pure-numpy emulation of the exact device computation (for harness-free
# validation of the host-side folding / layout / sharding logic)
def emulate(**inputs) -> np.ndarray:
    x = np.asarray(inputs["x"], np.float32)
    mask = np.asarray(inputs["mask"], np.float32)
    rsin = np.asarray(inputs["rope_sin"], np.float32)
    rcos = np.asarray(inputs["rope_cos"], np.float32)
    wqkv = np.asarray(inputs["wqkv"], np.float32)
    wo = np.asarray(inputs["wo"], np.float32)
    n1 = np.asarray(inputs["norm1_w"], np.float32)
    n2 = np.asarray(inputs["norm2_w"], np.float32)
    wk = np.asarray(inputs["ffn_wk"], np.float32)
    wv = np.asarray(inputs["ffn_wv"], np.float32)

    wqkv_f = wqkv * n1[:, None]
    wk_f = wk * n2[:, None]
    scale = HD ** -0.5
    R = np.zeros((HD, HD), np.float32)
    idx = np.arange(0, HD, 2)
    R[idx, idx + 1] = -1.0
    R[idx + 1, idx] = 1.0
    with np.errstate(under="ignore", over="ignore"):
        emfull = np.exp(mask).astype(np.float32)

    y = np.empty((B, S, D), np.float32)
    # per batch: compute k/v for all tokens (mimics the AllGather result)
    for bc in range(B):
        xb = x[bc]                                     # [S, D]
        xn = xb * (1.0 / np.sqrt((xb ** 2).mean(-1, keepdims=True) + EPS))
        qkv = xn @ wqkv_f                              # [S, NFT*128]
        sinT, cosT = rsin.T, rcos.T                    # [HD, S]
        k_all = np.empty((KVC, HD, S), np.float32)
        v_all = np.empty((KVC, S, HD), np.float32)
        for h in range(KVC):
            kh = qkv[:, (16 + h) * 128:(17 + h) * 128].T   # [HD, S]
            kh = kh * (cosT * scale) + (R @ kh) * (sinT * scale)
            k_all[h] = kh
            v_all[h] = qkv[:, (20 + h) * 128:(21 + h) * 128]
        for sc in range(4):
            tok = slice(T * sc, T * (sc + 1))
            o_heads = np.empty((QC, HD, T), np.float32)
            for h in range(QC):
                qh = qkv[tok, h * 128:(h + 1) * 128].T     # [HD, T]
                qh = qh * cosT[:, tok] + (R @ qh) * sinT[:, tok]
                sT = k_all[h % KVC].T @ qh                 # [S, T] scores.T
                eT = np.exp(sT) * emfull[tok, :].T         # [S(tk), T(tq)]
                den = eT.sum(0)
                o_heads[h] = (v_all[h % KVC].T @ eT) / den
            o_fm = o_heads.reshape(D, T)                   # [(h,hd), T]
            y1 = xb[tok] + (o_fm.T @ wo)                   # [T, D]
            xn2 = y1 * (1.0 / np.sqrt((y1 ** 2).mean(-1, keepdims=True) + EPS))
            hh = np.maximum(xn2 @ wk_f, 0.0) ** 2
            y[bc, tok] = y1 + hh @ wv
    return y
